# revision 10
# baseline (speedup 1.0000x reference)
"""FFT-based DCT-II on 8 trn2 NeuronCores — stream-transpose design (v3).

Per core (256 rows): Makhoul DCT->real-FFT, four-step radix-64x64.
Stage 1 uses a block-diagonal [128,128] stationary (two 64-pt real-DFT
blocks for the n2-parity halves) so all 128 PE rows contract at once.
Mid-transpose pipeline per r-group chunk g:
  psum --scalar drain/cast--> t(fp16) --DVE 32x32 stream-transpose-->
  tr (natural block layout) --gpsimd free-permute--> tt2 (jl-major,
  contiguous stage-2 moving slices)
Stage 2 is split into r-halves: the first half's matmuls overlap the
second half's transposes/compactions so the PE never idles long.
fp16 operands, fp32 psum, fp16 output (cast to fp32 on host).

Layouts (per core):
  x1 [128 p=(n2_0, n1), 8192 f=(r 256, nl 32)]  n = 64*n1 + n2, n2 = 2*nl + n2_0
  f1 [128, 128] block-diag; slots j: 0..32 cos a, 33..63 sin a=j-32
  tt2A/B [128 p2=(n2_0, jg, nl), 32 a, 128 r-half]: jg=0 slot a = Re[a];
     jg=1 slot 0 = Re[32], slot a>=1 = Im[a]
  hh [128 p2, 32 a, 128 po=(d,k2)]; slice 0 embeds the k1=0 / k1=32
     tables in disjoint quadrants (zeros elsewhere)
  y  [128 po, 32 slice, 256 r] fp16
"""

import numpy as np

N = 4096
R = 2048
RPC = 256

_state = {}


def _tables():
    n1 = np.arange(64)[:, None].astype(np.float64)
    a33 = np.arange(33)[None, :].astype(np.float64)
    cos = np.cos(2 * np.pi * n1 * a33 / 64)
    sin = -np.sin(2 * np.pi * n1 * a33[:, 1:32] / 64)
    F1 = np.concatenate([cos, sin], axis=1)  # [64, 64]
    f1 = np.zeros((128, 128), dtype=np.float64)
    for c in range(2):
        f1[c * 64 : (c + 1) * 64, c * 64 : (c + 1) * 64] = F1
    f1_np = np.ascontiguousarray(f1.astype(np.float16))

    n2v = np.arange(64)[:, None].astype(np.float64)
    k2v = np.arange(64)[None, :].astype(np.float64)

    def HH_single(k1):
        k = 64 * k2v + k1
        Gc = np.cos(2 * np.pi * n2v * k / N)
        Gs = -np.sin(2 * np.pi * n2v * k / N)
        cosE = np.cos(np.pi * k / (2 * N))
        sinE = np.sin(np.pi * k / (2 * N))
        sigma = 1.0 if k1 <= 32 else -1.0
        H1 = cosE * Gc + sinE * Gs
        H2 = sigma * (sinE * Gc - cosE * Gs)
        return H1, H2  # [64 n2, 64 k2] each

    def rows(H, n2_0):
        return H[2 * np.arange(32) + n2_0, :]

    hh = np.zeros((128, 32, 128), dtype=np.float64)  # [p2, a, po]
    for a in range(1, 32):
        H1a, H2a = HH_single(a)
        H1b, H2b = HH_single(64 - a)
        for n2_0 in range(2):
            b = n2_0 * 64
            hh[b : b + 32, a, 0:64] = rows(H1a, n2_0)
            hh[b + 32 : b + 64, a, 0:64] = rows(H2a, n2_0)
            hh[b : b + 32, a, 64:128] = rows(H1b, n2_0)
            hh[b + 32 : b + 64, a, 64:128] = rows(H2b, n2_0)
    H10, _ = HH_single(0)
    H132, _ = HH_single(32)
    for n2_0 in range(2):
        b = n2_0 * 64
        hh[b : b + 32, 0, 0:64] = rows(H10, n2_0)
        hh[b + 32 : b + 64, 0, 64:128] = rows(H132, n2_0)
    hh_np = np.ascontiguousarray(hh.astype(np.float16))
    return f1_np, hh_np


def _build():
    import concourse.tile as tile
    from concourse import bacc, mybir

    f16 = mybir.dt.float16
    f32 = mybir.dt.float32

    nc = bacc.Bacc("TRN2", target_bir_lowering=False, debug=False, num_devices=8)
    x1_d = nc.dram_tensor("x1", [128, 8192], f16, kind="ExternalInput").ap()
    f1_d = nc.dram_tensor("f1", [128, 128], f16, kind="ExternalInput").ap()
    hh_d = nc.dram_tensor("hh", [128, 32, 128], f16, kind="ExternalInput").ap()
    y_d = nc.dram_tensor("y", [128, 32, 256], f16, kind="ExternalOutput").ap()

    with tile.TileContext(nc) as tc:
        with (
            tc.tile_pool(name="const", bufs=1) as const,
            tc.tile_pool(name="data", bufs=1) as data,
            tc.tile_pool(name="xp", bufs=4) as xp,
            tc.tile_pool(name="tp", bufs=3) as tp,
            tc.tile_pool(name="trp", bufs=3) as trp,
            tc.tile_pool(name="ps1", bufs=2, space="PSUM") as ps1,
            tc.tile_pool(name="ps2", bufs=6, space="PSUM") as ps2,
        ):
            f1_sb = const.tile([128, 128], f16)
            hh_sb = const.tile([128, 32, 128], f16)
            nc.sync.dma_start(f1_sb[:], f1_d)

            x1_c = []
            for c in range(8):
                xg = xp.tile([128, 1024], f16, name=f"x1_{c}")
                nc.sync.dma_start(xg[:], x1_d[:, 1024 * c : 1024 * c + 1024])
                x1_c.append(xg)
            nc.scalar.dma_start(hh_sb[:, 0:16, :], hh_d[:, 0:16, :])
            nc.scalar.dma_start(hh_sb[:, 16:32, :], hh_d[:, 16:32, :])

            # jl-major compacted T, split by r-half for early stage-2 start
            tt2 = [data.tile([128, 32, 128], f16, name=f"tt2_{h}") for h in range(2)]

            # stage 1 + transpose + compaction pipeline
            for g in range(16):
                ps = ps1.tile([128, 512], f32, name=f"s1_{g}", tag="s1")
                nc.tensor.matmul(
                    ps[:],
                    f1_sb[:],
                    x1_c[g // 2][:, 512 * (g % 2) : 512 * (g % 2) + 512],
                    start=True,
                    stop=True,
                )
                tg = tp.tile([128, 16, 32], f16, name=f"t_{g % 3}")
                nc.scalar.copy(tg[:], ps[:].rearrange("p (r l) -> p r l", l=32))
                tr = trp.tile([128, 16, 32], f16, name=f"tr_{g % 3}")
                nc.vector.transpose(tr[:], tg[:])
                dst = tt2[g // 8][:, :, 16 * (g % 8) : 16 * (g % 8) + 16]
                nc.gpsimd.tensor_copy(dst, tr[:].rearrange("p r l -> p l r"))

            # stage 2, r-half phases; slice 0's k1=0 / k1=32 tables live in
            # disjoint quadrants of hh[:, 0, :] (zeros elsewhere)
            y_sb = [
                data.tile([128, 4, 256], f16, name=f"y_{t}") for t in range(8)
            ]
            for h in range(2):
                for q in range(16):
                    ps = ps2.tile([128, 2, 128], f32, name=f"s2_{h}_{q}", tag="s2")
                    for i in range(2):
                        a = 2 * q + i
                        nc.tensor.matmul(
                            ps[:, i, :], hh_sb[:, a, :], tt2[h][:, a, :],
                            start=True, stop=True,
                        )
                    dst = y_sb[q // 2][:, 2 * (q % 2) : 2 * (q % 2) + 2,
                                       128 * h : 128 * h + 128]
                    if h == 1 and q % 2 == 1:
                        nc.vector.tensor_copy(dst, ps[:])
                        nc.sync.dma_start(
                            y_d[:, 4 * (q // 2) : 4 * (q // 2) + 4, :],
                            y_sb[q // 2][:],
                        )
                    else:
                        nc.scalar.copy(dst, ps[:])

    nc.compile()
    return nc


def _pack_x1(x_rows):
    v = np.empty_like(x_rows)
    v[:, : N // 2] = x_rows[:, 0::2]
    v[:, N // 2 :] = x_rows[:, 1::2][:, ::-1]
    v = v.reshape(RPC, 64, 32, 2)  # [r, n1, nl, n2_0]
    x1 = v.transpose(3, 1, 0, 2).reshape(128, RPC * 32)
    return np.ascontiguousarray(x1.astype(np.float16))


def kernel(x, _trace: bool = False):
    from concourse.bass_utils import run_bass_kernel_spmd

    x = np.asarray(x, dtype=np.float32)
    assert x.shape == (R, N)
    if "nc" not in _state:
        _state["nc"] = _build()
        _state["tables"] = _tables()
    nc = _state["nc"]
    f1_np, hh_np = _state["tables"]

    in_maps = []
    for c in range(8):
        in_maps.append(
            {
                "x1": _pack_x1(x[c * RPC : (c + 1) * RPC]),
                "f1": f1_np,
                "hh": hh_np,
            }
        )

    res = run_bass_kernel_spmd(nc, in_maps, list(range(8)), trace=_trace)

    k2 = np.arange(64)
    y = np.empty((R, N), dtype=np.float32)
    for c in range(8):
        yf = res.results[c]["y"].astype(np.float32)  # [128 po, 32 slice, 256 r]
        out = y[c * RPC : (c + 1) * RPC]
        out[:, 64 * k2] = yf[0:64, 0, :].T
        out[:, 32 + 64 * k2] = yf[64:128, 0, :].T
        for a in range(1, 32):
            out[:, a + 64 * k2] = yf[0:64, a, :].T
            out[:, (64 - a) + 64 * k2] = yf[64:128, a, :].T
    if _trace:
        _state["last_result"] = res
    return y


# revision 11
# speedup vs baseline: 1.4832x; 1.4832x over previous
"""FFT-based DCT-II on 8 trn2 NeuronCores — stream-transpose + flipped
stage-2 (v4).

Per core (256 rows): Makhoul DCT->real-FFT, four-step radix-64x64.
Stage 1 uses a block-diagonal [128,128] stationary (two 64-pt real-DFT
blocks for the n2-parity halves) so all 128 PE rows contract at once.
Mid-transpose: per r-group chunk, psum is drained (scalar, fp32->fp16)
and DVE 32x32 stream-transposed into tt half-tiles in natural block
layout (free = (r, jl)).
Stage 2 is FLIPPED: stationary = tt[:, r-block, a] (strided columns —
free for LDWEIGHTS), moving = hh[:, a, :] (contiguous). Output lands
row-partitioned: psum [r-local, (d k2)]. Each flip unit needs only one
r-half, so rb=0 units overlap the rb=1 half's transposes.
fp16 operands, fp32 psum, fp16 output (cast to fp32 on host).

Layouts (per core):
  x1 [128 p=(n2_0, n1), 8192 f=(r 256, nl 32)]  n = 64*n1 + n2, n2 = 2*nl + n2_0
  f1 [128, 128] block-diag; slots j: 0..32 cos a, 33..63 sin a=j-32
  tt[h] [128 p2=(n2_0, jg, nl), 128 r-local, 32 jl]: jg=0 slot a = Re[a];
     jg=1 slot 0 = Re[32], slot a>=1 = Im[a]
  hh [128 p2, 32 a, 128 po=(d,k2)]; slice 0 embeds the k1=0 / k1=32
     tables in disjoint quadrants (zeros elsewhere)
  y  [128 rl, 2 rb, 32 a, 128 (d k2)] fp16
"""

import numpy as np

N = 4096
R = 2048
RPC = 256

_state = {}


def _tables():
    n1 = np.arange(64)[:, None].astype(np.float64)
    a33 = np.arange(33)[None, :].astype(np.float64)
    cos = np.cos(2 * np.pi * n1 * a33 / 64)
    sin = -np.sin(2 * np.pi * n1 * a33[:, 1:32] / 64)
    F1 = np.concatenate([cos, sin], axis=1)  # [64, 64]
    f1 = np.zeros((128, 128), dtype=np.float64)
    for c in range(2):
        f1[c * 64 : (c + 1) * 64, c * 64 : (c + 1) * 64] = F1
    f1_np = np.ascontiguousarray(f1.astype(np.float16))

    n2v = np.arange(64)[:, None].astype(np.float64)
    k2v = np.arange(64)[None, :].astype(np.float64)

    def HH_single(k1):
        k = 64 * k2v + k1
        Gc = np.cos(2 * np.pi * n2v * k / N)
        Gs = -np.sin(2 * np.pi * n2v * k / N)
        cosE = np.cos(np.pi * k / (2 * N))
        sinE = np.sin(np.pi * k / (2 * N))
        sigma = 1.0 if k1 <= 32 else -1.0
        H1 = cosE * Gc + sinE * Gs
        H2 = sigma * (sinE * Gc - cosE * Gs)
        return H1, H2  # [64 n2, 64 k2] each

    def rows(H, n2_0):
        return H[2 * np.arange(32) + n2_0, :]

    hh = np.zeros((128, 32, 128), dtype=np.float64)  # [p2, a, po]
    for a in range(1, 32):
        H1a, H2a = HH_single(a)
        H1b, H2b = HH_single(64 - a)
        for n2_0 in range(2):
            b = n2_0 * 64
            hh[b : b + 32, a, 0:64] = rows(H1a, n2_0)
            hh[b + 32 : b + 64, a, 0:64] = rows(H2a, n2_0)
            hh[b : b + 32, a, 64:128] = rows(H1b, n2_0)
            hh[b + 32 : b + 64, a, 64:128] = rows(H2b, n2_0)
    H10, _ = HH_single(0)
    H132, _ = HH_single(32)
    for n2_0 in range(2):
        b = n2_0 * 64
        hh[b : b + 32, 0, 0:64] = rows(H10, n2_0)
        hh[b + 32 : b + 64, 0, 64:128] = rows(H132, n2_0)
    hh_np = np.ascontiguousarray(hh.astype(np.float16))
    return f1_np, hh_np


def _build():
    import concourse.tile as tile
    from concourse import bacc, mybir

    f16 = mybir.dt.float16
    f32 = mybir.dt.float32

    nc = bacc.Bacc("TRN2", target_bir_lowering=False, debug=False, num_devices=8)
    x1_d = nc.dram_tensor("x1", [128, 8192], f16, kind="ExternalInput").ap()
    f1_d = nc.dram_tensor("f1", [128, 128], f16, kind="ExternalInput").ap()
    hh_d = nc.dram_tensor("hh", [128, 32, 128], f16, kind="ExternalInput").ap()
    y_d = nc.dram_tensor("y", [128, 2, 32, 128], f16, kind="ExternalOutput").ap()

    with tile.TileContext(nc) as tc:
        with (
            tc.tile_pool(name="const", bufs=1) as const,
            tc.tile_pool(name="data", bufs=1) as data,
            tc.tile_pool(name="xp", bufs=4) as xp,
            tc.tile_pool(name="tp", bufs=3) as tp,
            tc.tile_pool(name="ps1", bufs=2, space="PSUM") as ps1,
            tc.tile_pool(name="ps2", bufs=6, space="PSUM") as ps2,
        ):
            f1_sb = const.tile([128, 128], f16)
            hh_sb = const.tile([128, 32, 128], f16)
            nc.sync.dma_start(f1_sb[:], f1_d)

            x1_c = []
            for c in range(8):
                xg = xp.tile([128, 1024], f16, name=f"x1_{c}")
                nc.sync.dma_start(xg[:], x1_d[:, 1024 * c : 1024 * c + 1024])
                x1_c.append(xg)
            nc.scalar.dma_start(hh_sb[:, 0:16, :], hh_d[:, 0:16, :])
            nc.scalar.dma_start(hh_sb[:, 16:32, :], hh_d[:, 16:32, :])

            # tt in natural stream-transpose layout, one tile per r-half
            tt = [data.tile([128, 128, 32], f16, name=f"tt_{h}") for h in range(2)]

            # stage 1 + transpose pipeline
            for g in range(16):
                ps = ps1.tile([128, 512], f32, name=f"s1_{g}", tag="s1")
                nc.tensor.matmul(
                    ps[:],
                    f1_sb[:],
                    x1_c[g // 2][:, 512 * (g % 2) : 512 * (g % 2) + 512],
                    start=True,
                    stop=True,
                )
                tg = tp.tile([128, 16, 32], f16, name=f"t_{g % 3}")
                nc.scalar.copy(tg[:], ps[:].rearrange("p (r l) -> p r l", l=32))
                dst = tt[g // 8][:, 16 * (g % 8) : 16 * (g % 8) + 16, :]
                nc.vector.transpose(dst, tg[:])

            # stage 2, flipped operands, rb phases
            y_sb = [
                data.tile([128, 2, 2, 128], f16, name=f"y_{t}") for t in range(16)
            ]
            for rb in range(2):
                for q in range(16):
                    ps = ps2.tile([128, 2, 128], f32, name=f"s2_{rb}_{q}", tag="s2")
                    for i in range(2):
                        a = 2 * q + i
                        nc.tensor.matmul(
                            ps[:, i, :], tt[rb][:, :, a], hh_sb[:, a, :],
                            start=True, stop=True,
                        )
                    t = 8 * rb + q // 2
                    dst = y_sb[t][:, q % 2, :, :]
                    if q % 2 == 0:
                        nc.scalar.copy(dst, ps[:])
                    else:
                        nc.vector.tensor_copy(dst, ps[:])
                        nc.sync.dma_start(
                            y_d[:, rb, 4 * (q // 2) : 4 * (q // 2) + 4, :],
                            y_sb[t][:].rearrange("p u i k -> p (u i) k"),
                        )

    nc.compile()
    return nc


def _pack_x1(x_rows):
    v = np.empty_like(x_rows)
    v[:, : N // 2] = x_rows[:, 0::2]
    v[:, N // 2 :] = x_rows[:, 1::2][:, ::-1]
    v = v.reshape(RPC, 64, 32, 2)  # [r, n1, nl, n2_0]
    x1 = v.transpose(3, 1, 0, 2).reshape(128, RPC * 32)
    return np.ascontiguousarray(x1.astype(np.float16))


def kernel(x, _trace: bool = False):
    from concourse.bass_utils import run_bass_kernel_spmd

    x = np.asarray(x, dtype=np.float32)
    assert x.shape == (R, N)
    if "nc" not in _state:
        _state["nc"] = _build()
        _state["tables"] = _tables()
    nc = _state["nc"]
    f1_np, hh_np = _state["tables"]

    in_maps = []
    for c in range(8):
        in_maps.append(
            {
                "x1": _pack_x1(x[c * RPC : (c + 1) * RPC]),
                "f1": f1_np,
                "hh": hh_np,
            }
        )

    res = run_bass_kernel_spmd(nc, in_maps, list(range(8)), trace=_trace)

    k2 = np.arange(64)
    y = np.empty((R, N), dtype=np.float32)
    for c in range(8):
        # y_d [128 rl, 2 rb, 32 a, 128 (d k2)] -> rows r = 128*rb + rl
        yf = res.results[c]["y"].astype(np.float32)
        rows = yf.transpose(1, 0, 2, 3).reshape(RPC, 32, 2, 64)  # [r, a, d, k2]
        out = y[c * RPC : (c + 1) * RPC]
        out[:, 64 * k2] = rows[:, 0, 0, :]
        out[:, 32 + 64 * k2] = rows[:, 0, 1, :]
        for a in range(1, 32):
            out[:, a + 64 * k2] = rows[:, a, 0, :]
            out[:, (64 - a) + 64 * k2] = rows[:, a, 1, :]
    if _trace:
        _state["last_result"] = res
    return y


# revision 14
# speedup vs baseline: 1.6684x; 1.1248x over previous
"""FFT-based DCT-II on 8 trn2 NeuronCores — stream-transpose + flipped
stage-2 (v4).

Per core (256 rows): Makhoul DCT->real-FFT, four-step radix-64x64.
Stage 1 uses a block-diagonal [128,128] stationary (two 64-pt real-DFT
blocks for the n2-parity halves) so all 128 PE rows contract at once.
Mid-transpose: per r-group chunk, psum is drained (scalar, fp32->fp16)
and DVE 32x32 stream-transposed into tt half-tiles in natural block
layout (free = (r, jl)).
Stage 2 is FLIPPED: stationary = tt[:, r-block, a] (strided columns —
free for LDWEIGHTS), moving = hh[:, a, :] (contiguous). Output lands
row-partitioned: psum [r-local, (d k2)]. Each flip unit needs only one
r-half, so rb=0 units overlap the rb=1 half's transposes.
fp16 operands, fp32 psum, fp16 output (cast to fp32 on host).

Layouts (per core):
  x1 [128 p=(n2_0, n1), 8192 f=(r 256, nl 32)]  n = 64*n1 + n2, n2 = 2*nl + n2_0
  f1 [128, 128] block-diag; slots j: 0..32 cos a, 33..63 sin a=j-32
  tt[h] [128 p2=(n2_0, jg, nl), 128 r-local, 32 jl]: jg=0 slot a = Re[a];
     jg=1 slot 0 = Re[32], slot a>=1 = Im[a]
  hh [128 p2, 32 a, 128 po=(d,k2)]; slice 0 embeds the k1=0 / k1=32
     tables in disjoint quadrants (zeros elsewhere)
  y  [128 rl, 2 rb, 32 a, 128 (d k2)] fp16
"""

import numpy as np

N = 4096
R = 2048
RPC = 256

_state = {}


def _tables():
    n1 = np.arange(64)[:, None].astype(np.float64)
    a33 = np.arange(33)[None, :].astype(np.float64)
    cos = np.cos(2 * np.pi * n1 * a33 / 64)
    sin = -np.sin(2 * np.pi * n1 * a33[:, 1:32] / 64)
    F1 = np.concatenate([cos, sin], axis=1)  # [64, 64]
    f1 = np.zeros((128, 128), dtype=np.float64)
    for c in range(2):
        f1[c * 64 : (c + 1) * 64, c * 64 : (c + 1) * 64] = F1
    f1_np = np.ascontiguousarray(f1.astype(np.float16))

    n2v = np.arange(64)[:, None].astype(np.float64)
    k2v = np.arange(64)[None, :].astype(np.float64)

    def HH_single(k1):
        k = 64 * k2v + k1
        Gc = np.cos(2 * np.pi * n2v * k / N)
        Gs = -np.sin(2 * np.pi * n2v * k / N)
        cosE = np.cos(np.pi * k / (2 * N))
        sinE = np.sin(np.pi * k / (2 * N))
        sigma = 1.0 if k1 <= 32 else -1.0
        H1 = cosE * Gc + sinE * Gs
        H2 = sigma * (sinE * Gc - cosE * Gs)
        return H1, H2  # [64 n2, 64 k2] each

    def rows(H, n2_0):
        return H[2 * np.arange(32) + n2_0, :]

    hh = np.zeros((128, 32, 128), dtype=np.float64)  # [p2, a, po]
    for a in range(1, 32):
        H1a, H2a = HH_single(a)
        H1b, H2b = HH_single(64 - a)
        for n2_0 in range(2):
            b = n2_0 * 64
            hh[b : b + 32, a, 0:64] = rows(H1a, n2_0)
            hh[b + 32 : b + 64, a, 0:64] = rows(H2a, n2_0)
            hh[b : b + 32, a, 64:128] = rows(H1b, n2_0)
            hh[b + 32 : b + 64, a, 64:128] = rows(H2b, n2_0)
    H10, _ = HH_single(0)
    H132, _ = HH_single(32)
    for n2_0 in range(2):
        b = n2_0 * 64
        hh[b : b + 32, 0, 0:64] = rows(H10, n2_0)
        hh[b + 32 : b + 64, 0, 64:128] = rows(H132, n2_0)
    hh_np = np.ascontiguousarray(hh.astype(np.float16))
    return f1_np, hh_np


def _build():
    import concourse.tile as tile
    from concourse import bacc, mybir

    f16 = mybir.dt.float16
    f32 = mybir.dt.float32

    nc = bacc.Bacc("TRN2", target_bir_lowering=False, debug=False, num_devices=8)
    x1_d = nc.dram_tensor("x1", [128, 8192], f16, kind="ExternalInput").ap()
    f1_d = nc.dram_tensor("f1", [128, 128], f16, kind="ExternalInput").ap()
    hh_d = nc.dram_tensor("hh", [128, 32, 128], f16, kind="ExternalInput").ap()
    y_d = nc.dram_tensor("y", [128, 2, 32, 128], f16, kind="ExternalOutput").ap()

    with tile.TileContext(nc) as tc:
        with (
            tc.tile_pool(name="const", bufs=1) as const,
            tc.tile_pool(name="data", bufs=1) as data,
            tc.tile_pool(name="xp", bufs=4) as xp,
            tc.tile_pool(name="tp", bufs=3) as tp,
            tc.tile_pool(name="ps1", bufs=4, space="PSUM") as ps1,
            tc.tile_pool(name="ps2", bufs=4, space="PSUM") as ps2,
        ):
            f1_sb = const.tile([128, 128], f16)
            hh_sb = const.tile([128, 32, 128], f16)
            nc.sync.dma_start(f1_sb[:], f1_d)

            # x1 chunks then hh, all on the sync queue (FIFO keeps x1 first;
            # scalar engine stays DMA-free for its drain work)
            x1_c = []
            for c in range(8):
                xg = xp.tile([128, 1024], f16, name=f"x1_{c}")
                nc.sync.dma_start(xg[:], x1_d[:, 1024 * c : 1024 * c + 1024])
                x1_c.append(xg)
                if c == 5:
                    nc.sync.dma_start(hh_sb[:, 0:16, :], hh_d[:, 0:16, :])
            nc.sync.dma_start(hh_sb[:, 16:32, :], hh_d[:, 16:32, :])

            # tt in natural stream-transpose layout, one tile per r-half
            tt = [data.tile([128, 128, 32], f16, name=f"tt_{h}") for h in range(2)]

            # stage 1 + transpose pipeline
            for g in range(16):
                ps = ps1.tile([128, 512], f32, name=f"s1_{g}", tag="s1")
                nc.tensor.matmul(
                    ps[:],
                    f1_sb[:],
                    x1_c[g // 2][:, 512 * (g % 2) : 512 * (g % 2) + 512],
                    start=True,
                    stop=True,
                )
                tg = tp.tile([128, 16, 32], f16, name=f"t_{g % 3}")
                nc.scalar.copy(tg[:], ps[:].rearrange("p (r l) -> p r l", l=32))
                dst = tt[g // 8][:, 16 * (g % 8) : 16 * (g % 8) + 16, :]
                nc.vector.transpose(dst, tg[:])

            # stage 2, flipped operands, rb phases; y shipped in 8 big DMAs
            # split across the sync and gpsimd (SW-DGE) queues
            y_sb = [
                data.tile([128, 4, 2, 128], f16, name=f"y_{t}") for t in range(8)
            ]
            for rb in range(2):
                for q in range(16):
                    ps = ps2.tile([128, 2, 128], f32, name=f"s2_{rb}_{q}", tag="s2")
                    for i in range(2):
                        a = 2 * q + i
                        nc.tensor.matmul(
                            ps[:, i, :], tt[rb][:, :, a], hh_sb[:, a, :],
                            start=True, stop=True,
                        )
                    t = 4 * rb + q // 4
                    dst = y_sb[t][:, q % 4, :, :]
                    if q % 2 == 0:
                        nc.scalar.copy(dst, ps[:])
                    else:
                        nc.vector.tensor_copy(dst, ps[:])
                    if q % 4 == 3:
                        dma_eng = nc.sync if t % 2 == 0 else nc.gpsimd
                        dma_eng.dma_start(
                            y_d[:, rb, 8 * (q // 4) : 8 * (q // 4) + 8, :],
                            y_sb[t][:].rearrange("p u i k -> p (u i) k"),
                        )

    nc.compile()
    return nc


def _pack_x1(x_rows):
    v = np.empty_like(x_rows)
    v[:, : N // 2] = x_rows[:, 0::2]
    v[:, N // 2 :] = x_rows[:, 1::2][:, ::-1]
    v = v.reshape(RPC, 64, 32, 2)  # [r, n1, nl, n2_0]
    x1 = v.transpose(3, 1, 0, 2).reshape(128, RPC * 32)
    return np.ascontiguousarray(x1.astype(np.float16))


def kernel(x, _trace: bool = False):
    from concourse.bass_utils import run_bass_kernel_spmd

    x = np.asarray(x, dtype=np.float32)
    assert x.shape == (R, N)
    if "nc" not in _state:
        _state["nc"] = _build()
        _state["tables"] = _tables()
    nc = _state["nc"]
    f1_np, hh_np = _state["tables"]

    in_maps = []
    for c in range(8):
        in_maps.append(
            {
                "x1": _pack_x1(x[c * RPC : (c + 1) * RPC]),
                "f1": f1_np,
                "hh": hh_np,
            }
        )

    res = run_bass_kernel_spmd(nc, in_maps, list(range(8)), trace=_trace)

    k2 = np.arange(64)
    y = np.empty((R, N), dtype=np.float32)
    for c in range(8):
        # y_d [128 rl, 2 rb, 32 a, 128 (d k2)] -> rows r = 128*rb + rl
        yf = res.results[c]["y"].astype(np.float32)
        rows = yf.transpose(1, 0, 2, 3).reshape(RPC, 32, 2, 64)  # [r, a, d, k2]
        out = y[c * RPC : (c + 1) * RPC]
        out[:, 64 * k2] = rows[:, 0, 0, :]
        out[:, 32 + 64 * k2] = rows[:, 0, 1, :]
        for a in range(1, 32):
            out[:, a + 64 * k2] = rows[:, a, 0, :]
            out[:, (64 - a) + 64 * k2] = rows[:, a, 1, :]
    if _trace:
        _state["last_result"] = res
    return y
